# revision 13
# baseline (speedup 1.0000x reference)
import sys, time

sys.path.insert(0, "/opt/trn_rl_repo")
import numpy as np
import ml_dtypes
from concourse import bass, bacc, tile, mybir
from concourse.bass_utils import run_bass_kernel_spmd

F32 = mybir.dt.float32
BF16 = mybir.dt.bfloat16
I32 = mybir.dt.int32
AL = mybir.AluOpType

B, N, DIM = 4, 2048, 1024
HEADS, DH = 16, 64
G = 8          # heads per core
GI = G * DH    # 512 = inner width per core
SCALE = DH ** -0.5
NB = N // 128   # 16 j-blocks
NCH = N // 512  # 4 q-chunks
DT = DIM // 128  # 8 dim tiles
IT = GI // 128   # 4 inner tiles

_CACHE = {}


def _build():
    nc = bacc.Bacc(None, target_bir_lowering=False)
    xT = nc.declare_dram_parameter("xT", [DIM, N], BF16, isOutput=False)
    wq = nc.declare_dram_parameter("wq", [DIM, GI], BF16, isOutput=False)
    wk = nc.declare_dram_parameter("wk", [DIM, GI], BF16, isOutput=False)
    wv = nc.declare_dram_parameter("wv", [DIM, GI], BF16, isOutput=False)
    wo = nc.declare_dram_parameter("wo", [GI, DIM], BF16, isOutput=False)
    msk = nc.declare_dram_parameter("msk", [128, 2048], BF16, isOutput=False)
    ese = nc.declare_dram_parameter("ese", [97, 256], BF16, isOutput=False)
    onv = nc.declare_dram_parameter("onv", [128, G], BF16, isOutput=False)
    out = nc.declare_dram_parameter("out", [DIM, N], BF16, isOutput=True)

    with tile.TileContext(nc) as tc:
        with (
            nc.allow_low_precision(reason="attention in bf16; rel-err gate 2e-2"),
            tc.tile_pool(name="big", bufs=1) as big,
            tc.tile_pool(name="pt", bufs=6) as ptp,
            tc.tile_pool(name="st", bufs=2) as stp,
            tc.tile_pool(name="ps", bufs=2, space="PSUM") as ps,
        ):
            # ---------------- persistent SBUF ----------------
            qT = [big.tile([128, N], BF16, name=f"qT{i}", tag=f"qT{i}") for i in range(IT)]
            kT = [big.tile([128, N], BF16, name=f"kT{i}", tag=f"kT{i}") for i in range(IT)]
            vg = [big.tile([128, G * (DH + 1)], BF16, name=f"v{r}", tag=f"v{r}") for r in range(NB)]
            mask = big.tile([128, 2048], BF16, name="mask", tag="mask")
            esel = big.tile([97, 256], BF16, name="esel", tag="esel")
            ot = [[big.tile([128, 512], BF16, name=f"ot{c}_{i}", tag=f"ot{c}_{i}") for i in range(IT)]
                  for c in range(NCH)]

            p1cm = tc.tile_pool(name="p1", bufs=1)
            p1 = p1cm.__enter__()
            xTb = [p1.tile([128, N], BF16, name=f"xT{d}", tag=f"xT{d}") for d in range(DT)]
            xTt = [[xTb[d][:, rc * 512:(rc + 1) * 512] for rc in range(4)] for d in range(DT)]
            wqb = p1.tile([128, DT * GI], BF16, name="wqb", tag="wqb")
            wkb = p1.tile([128, DT * GI], BF16, name="wkb", tag="wkb")
            wvb = p1.tile([128, DT * GI], BF16, name="wvb", tag="wvb")
            wqt = [wqb[:, d * GI:(d + 1) * GI] for d in range(DT)]
            wkt = [wkb[:, d * GI:(d + 1) * GI] for d in range(DT)]
            wvt = [wvb[:, d * GI:(d + 1) * GI] for d in range(DT)]
            wob = p1.tile([128, IT * DIM], BF16, name="wob", tag="wob")
            wot = [wob[:, i * DIM:(i + 1) * DIM] for i in range(IT)]

            # warm the ACT exp table + keep the PE busy (HAM warm) while the
            # first input DMAs land
            dmy = stp.tile([32, 32], F32, name="dmy", tag="dmy")
            nc.vector.memset(dmy[:], 0.0)
            nc.scalar.activation(dmy[:], dmy[:], mybir.ActivationFunctionType.Exp)
            dmw = p1.tile([128, 512], BF16, name="dmw", tag="dmw")
            nc.vector.memset(dmw[:], 0.0)
            for w in range(28):
                pw = ps.tile([128, 512], F32, name="pmix", tag="pmix")
                nc.tensor.matmul(pw[:], dmw[:, 0:128], dmw[:], start=True, stop=True)

            # DMAs: single coalesced descriptors, all on the sync queue so the
            # ACT queue stays free for compute. Critical-path order: wk, xT,
            # wv, wq, then the rest.
            nc.sync.dma_start(wkb[:].rearrange("p (d c) -> p d c", c=GI),
                              wk[:].rearrange("(d p) c -> p d c", p=128))
            for d in range(DT):
                nc.sync.dma_start(xTb[d][:], xT[d * 128:(d + 1) * 128, :])
            nc.sync.dma_start(wvb[:].rearrange("p (d c) -> p d c", c=GI),
                              wv[:].rearrange("(d p) c -> p d c", p=128))
            nc.sync.dma_start(wqb[:].rearrange("p (d c) -> p d c", c=GI),
                              wq[:].rearrange("(d p) c -> p d c", p=128))
            for r in range(4):
                dst = vg[r][:].rearrange("p (h c) -> p h c", c=DH + 1)[:, :, DH:DH + 1]
                nc.sync.dma_start(dst, onv[:].rearrange("p (h c) -> p h c", c=1))
            nc.sync.dma_start(mask[:], msk[:])
            nc.sync.dma_start(esel[:], ese[:])
            nc.sync.dma_start(wob[:].rearrange("p (i c) -> p i c", c=DIM),
                              wo[:].rearrange("(i p) c -> p i c", p=128))
            for r in range(4, NB):
                dst = vg[r][:].rearrange("p (h c) -> p h c", c=DH + 1)[:, :, DH:DH + 1]
                nc.sync.dma_start(dst, onv[:].rearrange("p (h c) -> p h c", c=1))

            # ---------------- projections ----------------
            evac_flip = [0]

            def evac2(dst, src):
                # alternate PSUM evacuations between DVE and ACT (phase 1 only)
                if evac_flip[0] % 2 == 0:
                    nc.vector.tensor_copy(dst, src)
                else:
                    nc.scalar.copy(dst, src)
                evac_flip[0] += 1

            def kproj_group(it, rc):
                pk = ps.tile([128, 512], F32, name="pmix", tag="pmix")
                for d in range(DT):
                    nc.tensor.matmul(pk[:], wkt[d][:, it * 128:(it + 1) * 128],
                                     xTt[d][rc], start=(d == 0), stop=(d == DT - 1))
                evac2(kT[it][:, rc * 512:(rc + 1) * 512], pk[:])

            def vproj_group(r):
                pv = ps.tile([128, 512], F32, name="pmix", tag="pmix")
                for d in range(DT):
                    nc.tensor.matmul(pv[:], xTb[d][:, r * 128:(r + 1) * 128],
                                     wvt[d], start=(d == 0), stop=(d == DT - 1))
                dst = vg[r][:].rearrange("p (h c) -> p h c", c=DH + 1)[:, :, 0:DH]
                if evac_flip[0] % 2 == 0:
                    nc.vector.tensor_copy(dst, pv[:].rearrange("p (h c) -> p h c", c=DH))
                else:
                    nc.scalar.copy(dst, pv[:].rearrange("p (h c) -> p h c", c=DH))
                evac_flip[0] += 1

            def qproj_group(it, rc, dve_only=False):
                pq = ps.tile([128, 512], F32, name="pmix", tag="pmix")
                for d in range(DT):
                    nc.tensor.matmul(pq[:], wqt[d][:, it * 128:(it + 1) * 128],
                                     xTt[d][rc], start=(d == 0), stop=(d == DT - 1))
                if dve_only:
                    nc.vector.tensor_copy(qT[it][:, rc * 512:(rc + 1) * 512], pq[:])
                else:
                    evac2(qT[it][:, rc * 512:(rc + 1) * 512], pq[:])

            # ---------------- out-projection (transposed) ----------------
            def outproj_group(ch, db):
                # out^T[db-block, i-chunk]: lhsT is the stable weight tile so
                # the PE's LDWEIGHTS pull-ahead never reads freshly-written data
                pf = ps.tile([128, 512], F32, name="pmix", tag="pmix")
                for i in range(IT):
                    nc.tensor.matmul(pf[:], wot[i][:, db * 128:(db + 1) * 128],
                                     ot[ch][i][:],
                                     start=(i == 0), stop=(i == IT - 1))
                so = stp.tile([128, 512], BF16, name="so", tag="so")
                if db % 2 == 0:
                    nc.vector.tensor_copy(so[:], pf[:])
                else:
                    nc.scalar.copy(so[:], pf[:])
                q_ = nc.sync if db % 2 == 0 else nc.scalar
                q_.dma_start(
                    out[db * 128:(db + 1) * 128,
                        ch * 512:(ch + 1) * 512], so[:])

            # ---------------- attention ----------------
            def attention_chunk(ch):
                ej = 4 * (ch + 1)
                ns = ej // 2
                d4a = stp.tile([97, 512], F32, name="d4", tag="d4")
                d4b = stp.tile([97, 512], F32, name="d4", tag="d4")
                nc.vector.memset(d4a[:], 1.0)
                nc.vector.memset(d4b[:], 1.0)
                d4 = [d4a, d4a, d4b, d4b]

                for hp in range(4):
                    hA, hB = 2 * hp, 2 * hp + 1
                    po0 = ps.tile([65, 512], F32, name="pot", tag="pot")
                    po1 = ps.tile([65, 512], F32, name="pot", tag="pot")

                    def s_mm(s):
                        # S^T pair for a 2-block supertile: row-tiled K=64 MMs
                        supA = ps.tile([128, 1024], F32, name="sup", tag="sup")
                        supB = ps.tile([128, 1024], F32, name="sup", tag="sup")
                        for half, jb in ((0, 2 * s), (1, 2 * s + 1)):
                            sl = slice(512 * half, 512 * half + 512)
                            nc.tensor.matmul(
                                supA[:, sl], kT[hp][0:64, jb * 128:(jb + 1) * 128],
                                qT[hp][0:64, ch * 512:(ch + 1) * 512])
                            nc.tensor.matmul(
                                supB[:, sl], kT[hp][64:128, jb * 128:(jb + 1) * 128],
                                qT[hp][64:128, ch * 512:(ch + 1) * 512])
                        return supA, supB

                    nxt = s_mm(0)
                    for s in range(ns):
                        supA, supB = nxt
                        ptA = ptp.tile([128, 1024], BF16, name="pt", tag="pt")
                        ptB = ptp.tile([128, 1024], BF16, name="pt", tag="pt")
                        nc.scalar.activation(ptA[:], supA[:],
                                             mybir.ActivationFunctionType.Exp)
                        nc.scalar.activation(ptB[:], supB[:],
                                             mybir.ActivationFunctionType.Exp)
                        if s + 1 < ns:
                            nxt = s_mm(s + 1)
                        if s >= ns - 2:  # diagonal supers: staircase mask
                            msl = slice(0, 1024) if s == ns - 2 else slice(1024, 2048)
                            nc.vector.tensor_mul(ptA[:], ptA[:], mask[:, msl])
                            nc.vector.tensor_mul(ptB[:], ptB[:], mask[:, msl])
                        for half, jb in ((0, 2 * s), (1, 2 * s + 1)):
                            sl = slice(512 * half, 512 * half + 512)
                            st_ = (s == 0 and half == 0)
                            sp_ = (s == ns - 1 and half == 1)
                            nc.tensor.matmul(
                                po0[0:65, :], vg[jb][:, hA * (DH + 1):(hA + 1) * (DH + 1)],
                                ptA[:, sl], start=st_, stop=sp_)
                            nc.tensor.matmul(
                                po1[0:65, :], vg[jb][:, hB * (DH + 1):(hB + 1) * (DH + 1)],
                                ptB[:, sl], start=st_, stop=sp_)

                    # D rows -> 32-aligned slots; O -> ot (unnormalized)
                    rA, rB = 32 * (hA % 4), 32 * (hB % 4)
                    nc.vector.tensor_copy(d4[hp][rA:rA + 1, :], po0[64:65, :])
                    nc.vector.tensor_copy(d4[hp][rB:rB + 1, :], po1[64:65, :])
                    nc.vector.tensor_copy(ot[ch][hp][0:64, :], po0[0:64, :])
                    nc.vector.tensor_copy(ot[ch][hp][64:128, :], po1[0:64, :])
                    if ch + 1 < NCH:
                        qproj_group(hp, ch + 1, dve_only=True)

                # ---- batched norm for this chunk ----
                for x in range(2):
                    dd = d4a if x == 0 else d4b
                    y0 = stp.tile([97, 512], F32, name="y0", tag="y0")
                    nc.vector.reciprocal_approx_fast(y0[:], dd[:])
                    rec = stp.tile([97, 512], BF16, name="rec", tag="rec")
                    nc.vector.tensor_copy(rec[:], y0[:])
                    for t in range(2):
                        ti = 2 * x + t
                        prep = ps.tile([128, 512], F32, name="pmix", tag="pmix")
                        nc.tensor.matmul(prep[:], esel[:, 128 * t:128 * (t + 1)],
                                         rec[:], start=True, stop=True)
                        nc.vector.tensor_mul(ot[ch][ti][:], ot[ch][ti][:], prep[:])

                # out-proj: filler for the next chunk's ACT-bound attention
                for db in range(DT):
                    outproj_group(ch, db)

            # phase 1 minimal prefix: everything attention ch0 needs
            for it in range(IT):
                kproj_group(it, 0)
            for r in range(4):
                vproj_group(r)
            for it in range(IT):
                qproj_group(it, 0)

            attention_chunk(0)

            # remaining projections: emitted after ch0 so they fill its
            # ACT-bound windows; all complete before the chunks that need them
            for rc in range(1, 4):
                for it in range(IT):
                    kproj_group(it, rc)
            for r in range(4, NB):
                vproj_group(r)

            for ch in range(1, NCH):
                attention_chunk(ch)

            p1cm.__exit__(None, None, None)

    nc.compile()
    return nc


def kernel(x, w_qkv, w_out, b_out):
    if "nc" not in _CACHE:
        _CACHE["nc"] = _build()
    nc = _CACHE["nc"]

    x = np.asarray(x, np.float32)
    w_qkv = np.asarray(w_qkv, np.float32)
    w_out = np.asarray(w_out, np.float32)
    b_out = np.asarray(b_out, np.float32)

    # staircase masks for the 4 diagonal block offsets:
    # mask_r[p, i] = 1 if p <= i - 128r ; layout [r0 | r1 | r2 | r3]
    p = np.arange(128)[:, None]
    i = np.arange(512)[None, :]
    msk2 = np.concatenate(
        [(p <= i - 128 * r).astype(np.float32) for r in range(4)], axis=1)

    # selector for denominator broadcast: [97, 256]
    ese = np.zeros((97, 256), np.float32)
    ese[0, 0:64] = 1.0      # even ti: head rows 0 (p<64), 32 (p>=64)
    ese[32, 64:128] = 1.0
    ese[64, 128:192] = 1.0  # odd ti: rows 64, 96
    ese[96, 192:256] = 1.0

    in_maps = []
    for c in range(8):
        b, g = c // 2, c % 2
        sl = slice(g * GI, (g + 1) * GI)
        in_maps.append(dict(
            xT=np.ascontiguousarray(x[b].T).astype(ml_dtypes.bfloat16),
            wq=(np.ascontiguousarray(w_qkv[:, sl]) * np.float32(SCALE)).astype(ml_dtypes.bfloat16),
            wk=np.ascontiguousarray(w_qkv[:, 1024 + g * GI:1024 + (g + 1) * GI]).astype(ml_dtypes.bfloat16),
            wv=np.ascontiguousarray(w_qkv[:, 2048 + g * GI:2048 + (g + 1) * GI]).astype(ml_dtypes.bfloat16),
            wo=np.ascontiguousarray(w_out[sl, :]).astype(ml_dtypes.bfloat16),
            msk=msk2.astype(ml_dtypes.bfloat16),
            ese=ese.astype(ml_dtypes.bfloat16),
            onv=np.ones((128, G), ml_dtypes.bfloat16),
        ))
    res = None
    for attempt in range(3):
        try:
            run_bass_kernel_spmd(nc, in_maps, core_ids=list(range(8)))  # warmup
            res = run_bass_kernel_spmd(nc, in_maps, core_ids=list(range(8)))
            break
        except Exception:
            if attempt == 2:
                raise
            time.sleep(10)
    _CACHE["res"] = res
    outs = [np.asarray(res.results[c]["out"], np.float32) for c in range(8)]
    full = np.empty((B, N, DIM), np.float32)
    for b in range(B):
        full[b] = (outs[2 * b] + outs[2 * b + 1]).T + b_out[None, :]
    return full


# revision 15
# speedup vs baseline: 1.1736x; 1.1736x over previous
import sys, time

sys.path.insert(0, "/opt/trn_rl_repo")
import numpy as np
import ml_dtypes
from concourse import bass, bacc, tile, mybir
from concourse.bass_utils import run_bass_kernel_spmd

F32 = mybir.dt.float32
BF16 = mybir.dt.bfloat16
I32 = mybir.dt.int32
AL = mybir.AluOpType

B, N, DIM = 4, 2048, 1024
HEADS, DH = 16, 64
G = 8          # heads per core
GI = G * DH    # 512 = inner width per core
SCALE = DH ** -0.5
NB = N // 128   # 16 j-blocks
NCH = N // 512  # 4 q-chunks
DT = DIM // 128  # 8 dim tiles
IT = GI // 128   # 4 inner tiles

_CACHE = {}


def _build():
    nc = bacc.Bacc(None, target_bir_lowering=False)
    xT = nc.declare_dram_parameter("xT", [DIM, N], BF16, isOutput=False)
    wq = nc.declare_dram_parameter("wq", [DIM, GI], BF16, isOutput=False)
    wk = nc.declare_dram_parameter("wk", [DIM, GI], BF16, isOutput=False)
    wv = nc.declare_dram_parameter("wv", [DIM, GI], BF16, isOutput=False)
    wo = nc.declare_dram_parameter("wo", [GI, DIM], BF16, isOutput=False)
    msk = nc.declare_dram_parameter("msk", [128, 2048], BF16, isOutput=False)
    ese = nc.declare_dram_parameter("ese", [97, 256], BF16, isOutput=False)
    onv = nc.declare_dram_parameter("onv", [128, G], BF16, isOutput=False)
    out = nc.declare_dram_parameter("out", [DIM, N], BF16, isOutput=True)

    with tile.TileContext(nc) as tc:
        with (
            nc.allow_low_precision(reason="attention in bf16; rel-err gate 2e-2"),
            tc.tile_pool(name="big", bufs=1) as big,
            tc.tile_pool(name="pt", bufs=6) as ptp,
            tc.tile_pool(name="st", bufs=2) as stp,
            tc.tile_pool(name="ps", bufs=2, space="PSUM") as ps,
        ):
            # ---------------- persistent SBUF ----------------
            qT = [big.tile([128, N], BF16, name=f"qT{i}", tag=f"qT{i}") for i in range(IT)]
            kT = [big.tile([128, N], BF16, name=f"kT{i}", tag=f"kT{i}") for i in range(IT)]
            vg = [big.tile([128, G * (DH + 1)], BF16, name=f"v{r}", tag=f"v{r}") for r in range(NB)]
            mask = big.tile([128, 2048], BF16, name="mask", tag="mask")
            esel = big.tile([97, 256], BF16, name="esel", tag="esel")
            ot = [[big.tile([128, 512], BF16, name=f"ot{c}_{i}", tag=f"ot{c}_{i}") for i in range(IT)]
                  for c in range(NCH)]

            p1cm = tc.tile_pool(name="p1", bufs=1)
            p1 = p1cm.__enter__()
            xTb = [p1.tile([128, N], BF16, name=f"xT{d}", tag=f"xT{d}") for d in range(DT)]
            xTt = [[xTb[d][:, rc * 512:(rc + 1) * 512] for rc in range(4)] for d in range(DT)]
            wqb = p1.tile([128, DT * GI], BF16, name="wqb", tag="wqb")
            wkb = p1.tile([128, DT * GI], BF16, name="wkb", tag="wkb")
            wvb = p1.tile([128, DT * GI], BF16, name="wvb", tag="wvb")
            wqt = [wqb[:, d * GI:(d + 1) * GI] for d in range(DT)]
            wkt = [wkb[:, d * GI:(d + 1) * GI] for d in range(DT)]
            wvt = [wvb[:, d * GI:(d + 1) * GI] for d in range(DT)]
            wob = p1.tile([128, IT * DIM], BF16, name="wob", tag="wob")
            wot = [wob[:, i * DIM:(i + 1) * DIM] for i in range(IT)]

            # warm the ACT exp table + keep the PE busy (HAM warm) while the
            # first input DMAs land
            dmy = stp.tile([32, 32], F32, name="dmy", tag="dmy")
            nc.vector.memset(dmy[:], 0.0)
            nc.scalar.activation(dmy[:], dmy[:], mybir.ActivationFunctionType.Exp)
            dmw = p1.tile([128, 512], BF16, name="dmw", tag="dmw")
            nc.vector.memset(dmw[:], 0.0)
            for w in range(28):
                pw = ps.tile([128, 512], F32, name="pmix", tag="pmix")
                nc.tensor.matmul(pw[:], dmw[:, 0:128], dmw[:], start=True, stop=True)

            # DMAs: single coalesced descriptors, all on the sync queue so the
            # ACT queue stays free for compute. Critical-path order: wk, xT,
            # wv, wq, then the rest.
            nc.sync.dma_start(wkb[:].rearrange("p (d c) -> p d c", c=GI),
                              wk[:].rearrange("(d p) c -> p d c", p=128))
            for d in range(DT):
                nc.sync.dma_start(xTb[d][:], xT[d * 128:(d + 1) * 128, :])
            nc.sync.dma_start(wvb[:].rearrange("p (d c) -> p d c", c=GI),
                              wv[:].rearrange("(d p) c -> p d c", p=128))
            nc.sync.dma_start(wqb[:].rearrange("p (d c) -> p d c", c=GI),
                              wq[:].rearrange("(d p) c -> p d c", p=128))
            for r in range(4):
                dst = vg[r][:].rearrange("p (h c) -> p h c", c=DH + 1)[:, :, DH:DH + 1]
                nc.sync.dma_start(dst, onv[:].rearrange("p (h c) -> p h c", c=1))
            nc.sync.dma_start(mask[:], msk[:])
            nc.sync.dma_start(esel[:], ese[:])
            nc.sync.dma_start(wob[:].rearrange("p (i c) -> p i c", c=DIM),
                              wo[:].rearrange("(i p) c -> p i c", p=128))
            for r in range(4, NB):
                dst = vg[r][:].rearrange("p (h c) -> p h c", c=DH + 1)[:, :, DH:DH + 1]
                nc.sync.dma_start(dst, onv[:].rearrange("p (h c) -> p h c", c=1))

            # ---------------- projections ----------------
            evac_flip = [0]

            def evac2(dst, src):
                # alternate PSUM evacuations between DVE and ACT (phase 1 only)
                if evac_flip[0] % 2 == 0:
                    nc.vector.tensor_copy(dst, src)
                else:
                    nc.scalar.copy(dst, src)
                evac_flip[0] += 1

            def kproj_group(it, rc, act_evac=False):
                pk = ps.tile([128, 512], F32, name="pmix", tag="pmix")
                for d in range(DT):
                    nc.tensor.matmul(pk[:], wkt[d][:, it * 128:(it + 1) * 128],
                                     xTt[d][rc], start=(d == 0), stop=(d == DT - 1))
                if act_evac:
                    # post-ch0 batches: ACT is idle at the chunk boundary while
                    # the DVE queue is backlogged -- keep these off the DVE
                    nc.scalar.copy(kT[it][:, rc * 512:(rc + 1) * 512], pk[:])
                else:
                    evac2(kT[it][:, rc * 512:(rc + 1) * 512], pk[:])

            def vproj_group(r):
                pv = ps.tile([128, 512], F32, name="pmix", tag="pmix")
                for d in range(DT):
                    nc.tensor.matmul(pv[:], xTb[d][:, r * 128:(r + 1) * 128],
                                     wvt[d], start=(d == 0), stop=(d == DT - 1))
                dst = vg[r][:].rearrange("p (h c) -> p h c", c=DH + 1)[:, :, 0:DH]
                nc.vector.tensor_copy(dst, pv[:].rearrange("p (h c) -> p h c", c=DH))

            def qproj_group(it, rc, dve_only=False):
                pq = ps.tile([128, 512], F32, name="pmix", tag="pmix")
                for d in range(DT):
                    nc.tensor.matmul(pq[:], wqt[d][:, it * 128:(it + 1) * 128],
                                     xTt[d][rc], start=(d == 0), stop=(d == DT - 1))
                if dve_only:
                    nc.vector.tensor_copy(qT[it][:, rc * 512:(rc + 1) * 512], pq[:])
                else:
                    evac2(qT[it][:, rc * 512:(rc + 1) * 512], pq[:])

            # ---------------- out-projection (transposed) ----------------
            def outproj_group(ch, db):
                # out^T[db-block, i-chunk]: lhsT is the stable weight tile so
                # the PE's LDWEIGHTS pull-ahead never reads freshly-written data
                pf = ps.tile([128, 512], F32, name="pmix", tag="pmix")
                for i in range(IT):
                    nc.tensor.matmul(pf[:], wot[i][:, db * 128:(db + 1) * 128],
                                     ot[ch][i][:],
                                     start=(i == 0), stop=(i == IT - 1))
                so = stp.tile([128, 512], BF16, name="so", tag="so")
                if db % 2 == 0:
                    nc.vector.tensor_copy(so[:], pf[:])
                else:
                    nc.scalar.copy(so[:], pf[:])
                q_ = nc.sync if db % 2 == 0 else nc.scalar
                q_.dma_start(
                    out[db * 128:(db + 1) * 128,
                        ch * 512:(ch + 1) * 512], so[:])

            # ---------------- attention ----------------
            def attention_chunk(ch):
                ej = 4 * (ch + 1)
                ns = ej // 2
                d4a = stp.tile([97, 512], F32, name="d4", tag="d4")
                d4b = stp.tile([97, 512], F32, name="d4", tag="d4")
                nc.vector.memset(d4a[:], 1.0)
                nc.vector.memset(d4b[:], 1.0)
                d4 = [d4a, d4a, d4b, d4b]

                for hp in range(4):
                    hA, hB = 2 * hp, 2 * hp + 1
                    po0 = ps.tile([65, 512], F32, name="pot", tag="pot")
                    po1 = ps.tile([65, 512], F32, name="pot", tag="pot")

                    def s_mm(s):
                        # S^T pair for a 2-block supertile: row-tiled K=64 MMs
                        supA = ps.tile([128, 1024], F32, name="sup", tag="sup")
                        supB = ps.tile([128, 1024], F32, name="sup", tag="sup")
                        for half, jb in ((0, 2 * s), (1, 2 * s + 1)):
                            sl = slice(512 * half, 512 * half + 512)
                            nc.tensor.matmul(
                                supA[:, sl], kT[hp][0:64, jb * 128:(jb + 1) * 128],
                                qT[hp][0:64, ch * 512:(ch + 1) * 512])
                            nc.tensor.matmul(
                                supB[:, sl], kT[hp][64:128, jb * 128:(jb + 1) * 128],
                                qT[hp][64:128, ch * 512:(ch + 1) * 512])
                        return supA, supB

                    nxt = s_mm(0)
                    for s in range(ns):
                        supA, supB = nxt
                        ptA = ptp.tile([128, 1024], BF16, name="pt", tag="pt")
                        ptB = ptp.tile([128, 1024], BF16, name="pt", tag="pt")
                        nc.scalar.activation(ptA[:], supA[:],
                                             mybir.ActivationFunctionType.Exp)
                        nc.scalar.activation(ptB[:], supB[:],
                                             mybir.ActivationFunctionType.Exp)
                        if s + 1 < ns:
                            nxt = s_mm(s + 1)
                        if s >= ns - 2:  # diagonal supers: staircase mask
                            msl = slice(0, 1024) if s == ns - 2 else slice(1024, 2048)
                            nc.vector.tensor_mul(ptA[:], ptA[:], mask[:, msl])
                            nc.vector.tensor_mul(ptB[:], ptB[:], mask[:, msl])
                        for half, jb in ((0, 2 * s), (1, 2 * s + 1)):
                            sl = slice(512 * half, 512 * half + 512)
                            st_ = (s == 0 and half == 0)
                            sp_ = (s == ns - 1 and half == 1)
                            nc.tensor.matmul(
                                po0[0:65, :], vg[jb][:, hA * (DH + 1):(hA + 1) * (DH + 1)],
                                ptA[:, sl], start=st_, stop=sp_)
                            nc.tensor.matmul(
                                po1[0:65, :], vg[jb][:, hB * (DH + 1):(hB + 1) * (DH + 1)],
                                ptB[:, sl], start=st_, stop=sp_)

                    # D rows -> 32-aligned slots; O -> ot (unnormalized)
                    rA, rB = 32 * (hA % 4), 32 * (hB % 4)
                    nc.vector.tensor_copy(d4[hp][rA:rA + 1, :], po0[64:65, :])
                    nc.vector.tensor_copy(d4[hp][rB:rB + 1, :], po1[64:65, :])
                    nc.vector.tensor_copy(ot[ch][hp][0:64, :], po0[0:64, :])
                    nc.vector.tensor_copy(ot[ch][hp][64:128, :], po1[0:64, :])
                    if ch + 1 < NCH:
                        qproj_group(hp, ch + 1, dve_only=True)

                # ---- batched norm for this chunk ----
                for x in range(2):
                    dd = d4a if x == 0 else d4b
                    y0 = stp.tile([97, 512], F32, name="y0", tag="y0")
                    nc.vector.reciprocal_approx_fast(y0[:], dd[:])
                    rec = stp.tile([97, 512], BF16, name="rec", tag="rec")
                    nc.vector.tensor_copy(rec[:], y0[:])
                    for t in range(2):
                        ti = 2 * x + t
                        prep = ps.tile([128, 512], F32, name="pmix", tag="pmix")
                        nc.tensor.matmul(prep[:], esel[:, 128 * t:128 * (t + 1)],
                                         rec[:], start=True, stop=True)
                        nc.vector.tensor_mul(ot[ch][ti][:], ot[ch][ti][:], prep[:])

                # out-proj: filler for the next chunk's ACT-bound attention
                for db in range(DT):
                    outproj_group(ch, db)

            # phase 1 minimal prefix: everything attention ch0 needs
            for it in range(IT):
                kproj_group(it, 0)
            for r in range(4):
                vproj_group(r)
            for it in range(IT):
                qproj_group(it, 0)

            attention_chunk(0)

            # remaining projections: emitted after ch0 so they fill its
            # ACT-bound windows; all complete before the chunks that need them
            for rc in range(1, 4):
                for it in range(IT):
                    kproj_group(it, rc, act_evac=True)
            for r in range(4, NB):
                vproj_group(r)

            for ch in range(1, NCH):
                attention_chunk(ch)

            p1cm.__exit__(None, None, None)

    nc.compile()
    return nc


def kernel(x, w_qkv, w_out, b_out):
    if "nc" not in _CACHE:
        _CACHE["nc"] = _build()
    nc = _CACHE["nc"]

    x = np.asarray(x, np.float32)
    w_qkv = np.asarray(w_qkv, np.float32)
    w_out = np.asarray(w_out, np.float32)
    b_out = np.asarray(b_out, np.float32)

    # staircase masks for the 4 diagonal block offsets:
    # mask_r[p, i] = 1 if p <= i - 128r ; layout [r0 | r1 | r2 | r3]
    p = np.arange(128)[:, None]
    i = np.arange(512)[None, :]
    msk2 = np.concatenate(
        [(p <= i - 128 * r).astype(np.float32) for r in range(4)], axis=1)

    # selector for denominator broadcast: [97, 256]
    ese = np.zeros((97, 256), np.float32)
    ese[0, 0:64] = 1.0      # even ti: head rows 0 (p<64), 32 (p>=64)
    ese[32, 64:128] = 1.0
    ese[64, 128:192] = 1.0  # odd ti: rows 64, 96
    ese[96, 192:256] = 1.0

    in_maps = []
    for c in range(8):
        b, g = c // 2, c % 2
        sl = slice(g * GI, (g + 1) * GI)
        in_maps.append(dict(
            xT=np.ascontiguousarray(x[b].T).astype(ml_dtypes.bfloat16),
            wq=(np.ascontiguousarray(w_qkv[:, sl]) * np.float32(SCALE)).astype(ml_dtypes.bfloat16),
            wk=np.ascontiguousarray(w_qkv[:, 1024 + g * GI:1024 + (g + 1) * GI]).astype(ml_dtypes.bfloat16),
            wv=np.ascontiguousarray(w_qkv[:, 2048 + g * GI:2048 + (g + 1) * GI]).astype(ml_dtypes.bfloat16),
            wo=np.ascontiguousarray(w_out[sl, :]).astype(ml_dtypes.bfloat16),
            msk=msk2.astype(ml_dtypes.bfloat16),
            ese=ese.astype(ml_dtypes.bfloat16),
            onv=np.ones((128, G), ml_dtypes.bfloat16),
        ))
    res = None
    for attempt in range(3):
        try:
            run_bass_kernel_spmd(nc, in_maps, core_ids=list(range(8)))  # warmup
            res = run_bass_kernel_spmd(nc, in_maps, core_ids=list(range(8)))
            break
        except Exception:
            if attempt == 2:
                raise
            time.sleep(10)
    _CACHE["res"] = res
    outs = [np.asarray(res.results[c]["out"], np.float32) for c in range(8)]
    full = np.empty((B, N, DIM), np.float32)
    for b in range(B):
        full[b] = (outs[2 * b] + outs[2 * b + 1]).T + b_out[None, :]
    return full


# revision 17
# speedup vs baseline: 1.1753x; 1.0015x over previous
import sys, time

sys.path.insert(0, "/opt/trn_rl_repo")
import numpy as np
import ml_dtypes
from concourse import bass, bacc, tile, mybir
from concourse.bass_utils import run_bass_kernel_spmd

F32 = mybir.dt.float32
BF16 = mybir.dt.bfloat16
I32 = mybir.dt.int32
AL = mybir.AluOpType

B, N, DIM = 4, 2048, 1024
HEADS, DH = 16, 64
G = 8          # heads per core
GI = G * DH    # 512 = inner width per core
SCALE = DH ** -0.5
NB = N // 128   # 16 j-blocks
NCH = N // 512  # 4 q-chunks
DT = DIM // 128  # 8 dim tiles
IT = GI // 128   # 4 inner tiles

_CACHE = {}


def _build():
    nc = bacc.Bacc(None, target_bir_lowering=False)
    xT = nc.declare_dram_parameter("xT", [DIM, N], BF16, isOutput=False)
    wq = nc.declare_dram_parameter("wq", [DIM, GI], BF16, isOutput=False)
    wk = nc.declare_dram_parameter("wk", [DIM, GI], BF16, isOutput=False)
    wv = nc.declare_dram_parameter("wv", [DIM, GI], BF16, isOutput=False)
    wo = nc.declare_dram_parameter("wo", [GI, DIM], BF16, isOutput=False)
    msk = nc.declare_dram_parameter("msk", [128, 2048], BF16, isOutput=False)
    ese = nc.declare_dram_parameter("ese", [97, 256], BF16, isOutput=False)
    onv = nc.declare_dram_parameter("onv", [128, G], BF16, isOutput=False)
    out = nc.declare_dram_parameter("out", [DIM, N], BF16, isOutput=True)

    with tile.TileContext(nc) as tc:
        with (
            nc.allow_low_precision(reason="attention in bf16; rel-err gate 2e-2"),
            tc.tile_pool(name="big", bufs=1) as big,
            tc.tile_pool(name="pt", bufs=4) as ptp,
            tc.tile_pool(name="st", bufs=2) as stp,
            tc.tile_pool(name="ps", bufs=2, space="PSUM") as ps,
        ):
            # ---------------- persistent SBUF ----------------
            qT = [big.tile([128, N], BF16, name=f"qT{i}", tag=f"qT{i}") for i in range(IT)]
            kT = [big.tile([128, N], BF16, name=f"kT{i}", tag=f"kT{i}") for i in range(IT)]
            vg = [big.tile([128, G * (DH + 1)], BF16, name=f"v{r}", tag=f"v{r}") for r in range(NB)]
            mask = big.tile([128, 2048], BF16, name="mask", tag="mask")
            esel = big.tile([97, 256], BF16, name="esel", tag="esel")
            ot = [[big.tile([128, 512], BF16, name=f"ot{c}_{i}", tag=f"ot{c}_{i}") for i in range(IT)]
                  for c in range(NCH)]

            p1cm = tc.tile_pool(name="p1", bufs=1)
            p1 = p1cm.__enter__()
            xTb = [p1.tile([128, N], BF16, name=f"xT{d}", tag=f"xT{d}") for d in range(DT)]
            xTt = [[xTb[d][:, rc * 512:(rc + 1) * 512] for rc in range(4)] for d in range(DT)]
            wqb = p1.tile([128, DT * GI], BF16, name="wqb", tag="wqb")
            wkb = p1.tile([128, DT * GI], BF16, name="wkb", tag="wkb")
            wvb = p1.tile([128, DT * GI], BF16, name="wvb", tag="wvb")
            wqt = [wqb[:, d * GI:(d + 1) * GI] for d in range(DT)]
            wkt = [wkb[:, d * GI:(d + 1) * GI] for d in range(DT)]
            wvt = [wvb[:, d * GI:(d + 1) * GI] for d in range(DT)]
            wob = p1.tile([128, IT * DIM], BF16, name="wob", tag="wob")
            wot = [wob[:, i * DIM:(i + 1) * DIM] for i in range(IT)]

            # warm the ACT exp table + keep the PE busy (HAM warm) while the
            # first input DMAs land
            dmy = stp.tile([32, 32], F32, name="dmy", tag="dmy")
            nc.vector.memset(dmy[:], 0.0)
            nc.scalar.activation(dmy[:], dmy[:], mybir.ActivationFunctionType.Exp)
            dmw = p1.tile([128, 512], BF16, name="dmw", tag="dmw")
            nc.vector.memset(dmw[:], 0.0)
            for w in range(28):
                pw = ps.tile([128, 512], F32, name="pmix", tag="pmix")
                nc.tensor.matmul(pw[:], dmw[:, 0:128], dmw[:], start=True, stop=True)

            # DMAs: single coalesced descriptors, all on the sync queue so the
            # ACT queue stays free for compute. Critical-path order: wk, xT,
            # wv, wq, then the rest.
            nc.sync.dma_start(wkb[:].rearrange("p (d c) -> p d c", c=GI),
                              wk[:].rearrange("(d p) c -> p d c", p=128))
            for d in range(DT):
                nc.sync.dma_start(xTb[d][:], xT[d * 128:(d + 1) * 128, :])
            nc.sync.dma_start(wvb[:].rearrange("p (d c) -> p d c", c=GI),
                              wv[:].rearrange("(d p) c -> p d c", p=128))
            nc.sync.dma_start(wqb[:].rearrange("p (d c) -> p d c", c=GI),
                              wq[:].rearrange("(d p) c -> p d c", p=128))
            for r in range(4):
                dst = vg[r][:].rearrange("p (h c) -> p h c", c=DH + 1)[:, :, DH:DH + 1]
                nc.sync.dma_start(dst, onv[:].rearrange("p (h c) -> p h c", c=1))
            nc.sync.dma_start(mask[:], msk[:])
            nc.sync.dma_start(esel[:], ese[:])
            nc.sync.dma_start(wob[:].rearrange("p (i c) -> p i c", c=DIM),
                              wo[:].rearrange("(i p) c -> p i c", p=128))
            for r in range(4, NB):
                dst = vg[r][:].rearrange("p (h c) -> p h c", c=DH + 1)[:, :, DH:DH + 1]
                nc.sync.dma_start(dst, onv[:].rearrange("p (h c) -> p h c", c=1))

            # ---------------- projections ----------------
            evac_flip = [0]

            def evac2(dst, src):
                # alternate PSUM evacuations between DVE and ACT (phase 1 only)
                if evac_flip[0] % 2 == 0:
                    nc.vector.tensor_copy(dst, src)
                else:
                    nc.scalar.copy(dst, src)
                evac_flip[0] += 1

            def kproj_group(it, rc):
                pk = ps.tile([128, 512], F32, name="pmix", tag="pmix")
                for d in range(DT):
                    nc.tensor.matmul(pk[:], wkt[d][:, it * 128:(it + 1) * 128],
                                     xTt[d][rc], start=(d == 0), stop=(d == DT - 1))
                evac2(kT[it][:, rc * 512:(rc + 1) * 512], pk[:])

            def vproj_group(r):
                pv = ps.tile([128, 512], F32, name="pmix", tag="pmix")
                for d in range(DT):
                    nc.tensor.matmul(pv[:], xTb[d][:, r * 128:(r + 1) * 128],
                                     wvt[d], start=(d == 0), stop=(d == DT - 1))
                dst = vg[r][:].rearrange("p (h c) -> p h c", c=DH + 1)[:, :, 0:DH]
                nc.vector.tensor_copy(dst, pv[:].rearrange("p (h c) -> p h c", c=DH))

            def qproj_group(it, rc, dve_only=False):
                pq = ps.tile([128, 512], F32, name="pmix", tag="pmix")
                for d in range(DT):
                    nc.tensor.matmul(pq[:], wqt[d][:, it * 128:(it + 1) * 128],
                                     xTt[d][rc], start=(d == 0), stop=(d == DT - 1))
                if dve_only:
                    nc.vector.tensor_copy(qT[it][:, rc * 512:(rc + 1) * 512], pq[:])
                else:
                    evac2(qT[it][:, rc * 512:(rc + 1) * 512], pq[:])

            # ---------------- out-projection (transposed) ----------------
            def outproj_group(ch, db):
                # out^T[db-block, i-chunk]: lhsT is the stable weight tile so
                # the PE's LDWEIGHTS pull-ahead never reads freshly-written data
                pf = ps.tile([128, 512], F32, name="pmix", tag="pmix")
                for i in range(IT):
                    nc.tensor.matmul(pf[:], wot[i][:, db * 128:(db + 1) * 128],
                                     ot[ch][i][:],
                                     start=(i == 0), stop=(i == IT - 1))
                so = stp.tile([128, 512], BF16, name="so", tag="so")
                if db % 2 == 0:
                    nc.vector.tensor_copy(so[:], pf[:])
                else:
                    nc.scalar.copy(so[:], pf[:])
                q_ = nc.sync if db % 2 == 0 else nc.scalar
                q_.dma_start(
                    out[db * 128:(db + 1) * 128,
                        ch * 512:(ch + 1) * 512], so[:])

            # ---------------- attention ----------------
            def attention_chunk(ch, pre_pair=None):
                ej = 4 * (ch + 1)
                ns = ej // 2
                d4a = stp.tile([97, 512], F32, name="d4", tag="d4")
                d4b = stp.tile([97, 512], F32, name="d4", tag="d4")
                nc.vector.memset(d4a[:], 1.0)
                nc.vector.memset(d4b[:], 1.0)
                d4 = [d4a, d4a, d4b, d4b]

                for hp in range(4):
                    if pre_pair is not None:
                        pre_pair(hp)
                    hA, hB = 2 * hp, 2 * hp + 1
                    po0 = ps.tile([65, 512], F32, name="pot", tag="pot")
                    po1 = ps.tile([65, 512], F32, name="pot", tag="pot")

                    def s_mm(s):
                        # S^T pair for a 2-block supertile: row-tiled K=64 MMs
                        supA = ps.tile([128, 1024], F32, name="sup", tag="sup")
                        supB = ps.tile([128, 1024], F32, name="sup", tag="sup")
                        for half, jb in ((0, 2 * s), (1, 2 * s + 1)):
                            sl = slice(512 * half, 512 * half + 512)
                            nc.tensor.matmul(
                                supA[:, sl], kT[hp][0:64, jb * 128:(jb + 1) * 128],
                                qT[hp][0:64, ch * 512:(ch + 1) * 512])
                            nc.tensor.matmul(
                                supB[:, sl], kT[hp][64:128, jb * 128:(jb + 1) * 128],
                                qT[hp][64:128, ch * 512:(ch + 1) * 512])
                        return supA, supB

                    nxt = s_mm(0)
                    for s in range(ns):
                        supA, supB = nxt
                        ptA = ptp.tile([128, 1024], BF16, name="pt", tag="pt")
                        ptB = ptp.tile([128, 1024], BF16, name="pt", tag="pt")
                        nc.scalar.activation(ptA[:], supA[:],
                                             mybir.ActivationFunctionType.Exp)
                        nc.scalar.activation(ptB[:], supB[:],
                                             mybir.ActivationFunctionType.Exp)
                        if s + 1 < ns:
                            nxt = s_mm(s + 1)
                        if s >= ns - 2:  # diagonal supers: staircase mask
                            msl = slice(0, 1024) if s == ns - 2 else slice(1024, 2048)
                            nc.vector.tensor_mul(ptA[:], ptA[:], mask[:, msl])
                            nc.vector.tensor_mul(ptB[:], ptB[:], mask[:, msl])
                        for half, jb in ((0, 2 * s), (1, 2 * s + 1)):
                            sl = slice(512 * half, 512 * half + 512)
                            st_ = (s == 0 and half == 0)
                            sp_ = (s == ns - 1 and half == 1)
                            nc.tensor.matmul(
                                po0[0:65, :], vg[jb][:, hA * (DH + 1):(hA + 1) * (DH + 1)],
                                ptA[:, sl], start=st_, stop=sp_)
                            nc.tensor.matmul(
                                po1[0:65, :], vg[jb][:, hB * (DH + 1):(hB + 1) * (DH + 1)],
                                ptB[:, sl], start=st_, stop=sp_)

                    # D rows -> 32-aligned slots; O -> ot (unnormalized)
                    rA, rB = 32 * (hA % 4), 32 * (hB % 4)
                    nc.vector.tensor_copy(d4[hp][rA:rA + 1, :], po0[64:65, :])
                    nc.vector.tensor_copy(d4[hp][rB:rB + 1, :], po1[64:65, :])
                    nc.vector.tensor_copy(ot[ch][hp][0:64, :], po0[0:64, :])
                    nc.vector.tensor_copy(ot[ch][hp][64:128, :], po1[0:64, :])
                    if ch + 1 < NCH:
                        qproj_group(hp, ch + 1, dve_only=True)

                # ---- batched norm for this chunk ----
                for x in range(2):
                    dd = d4a if x == 0 else d4b
                    y0 = stp.tile([97, 512], F32, name="y0", tag="y0")
                    nc.vector.reciprocal_approx_fast(y0[:], dd[:])
                    rec = stp.tile([97, 512], BF16, name="rec", tag="rec")
                    nc.vector.tensor_copy(rec[:], y0[:])
                    for t in range(2):
                        ti = 2 * x + t
                        prep = ps.tile([128, 512], F32, name="pmix", tag="pmix")
                        nc.tensor.matmul(prep[:], esel[:, 128 * t:128 * (t + 1)],
                                         rec[:], start=True, stop=True)
                        nc.vector.tensor_mul(ot[ch][ti][:], ot[ch][ti][:], prep[:])

                # out-proj: filler for the next chunk's ACT-bound attention
                for db in range(DT):
                    outproj_group(ch, db)

            # phase 1 minimal prefix: only what attention ch0 pair 0 needs;
            # later pairs' k/q projections are emitted just before each pair
            kproj_group(0, 0)
            for r in range(4):
                vproj_group(r)
            qproj_group(0, 0)

            def ch0_pre(hp):
                if hp > 0:
                    kproj_group(hp, 0)
                    qproj_group(hp, 0)

            attention_chunk(0, pre_pair=ch0_pre)

            # remaining projections: emitted after ch0 so they fill its
            # ACT-bound windows; all complete before the chunks that need them
            for rc in range(1, 4):
                for it in range(IT):
                    kproj_group(it, rc)
            for r in range(4, NB):
                vproj_group(r)

            for ch in range(1, NCH):
                attention_chunk(ch)

            p1cm.__exit__(None, None, None)

    nc.compile()
    return nc


def kernel(x, w_qkv, w_out, b_out):
    if "nc" not in _CACHE:
        _CACHE["nc"] = _build()
    nc = _CACHE["nc"]

    x = np.asarray(x, np.float32)
    w_qkv = np.asarray(w_qkv, np.float32)
    w_out = np.asarray(w_out, np.float32)
    b_out = np.asarray(b_out, np.float32)

    # staircase masks for the 4 diagonal block offsets:
    # mask_r[p, i] = 1 if p <= i - 128r ; layout [r0 | r1 | r2 | r3]
    p = np.arange(128)[:, None]
    i = np.arange(512)[None, :]
    msk2 = np.concatenate(
        [(p <= i - 128 * r).astype(np.float32) for r in range(4)], axis=1)

    # selector for denominator broadcast: [97, 256]
    ese = np.zeros((97, 256), np.float32)
    ese[0, 0:64] = 1.0      # even ti: head rows 0 (p<64), 32 (p>=64)
    ese[32, 64:128] = 1.0
    ese[64, 128:192] = 1.0  # odd ti: rows 64, 96
    ese[96, 192:256] = 1.0

    in_maps = []
    for c in range(8):
        b, g = c // 2, c % 2
        sl = slice(g * GI, (g + 1) * GI)
        in_maps.append(dict(
            xT=np.ascontiguousarray(x[b].T).astype(ml_dtypes.bfloat16),
            wq=(np.ascontiguousarray(w_qkv[:, sl]) * np.float32(SCALE)).astype(ml_dtypes.bfloat16),
            wk=np.ascontiguousarray(w_qkv[:, 1024 + g * GI:1024 + (g + 1) * GI]).astype(ml_dtypes.bfloat16),
            wv=np.ascontiguousarray(w_qkv[:, 2048 + g * GI:2048 + (g + 1) * GI]).astype(ml_dtypes.bfloat16),
            wo=np.ascontiguousarray(w_out[sl, :]).astype(ml_dtypes.bfloat16),
            msk=msk2.astype(ml_dtypes.bfloat16),
            ese=ese.astype(ml_dtypes.bfloat16),
            onv=np.ones((128, G), ml_dtypes.bfloat16),
        ))
    res = None
    for attempt in range(3):
        try:
            run_bass_kernel_spmd(nc, in_maps, core_ids=list(range(8)))  # warmup
            res = run_bass_kernel_spmd(nc, in_maps, core_ids=list(range(8)))
            break
        except Exception:
            if attempt == 2:
                raise
            time.sleep(10)
    _CACHE["res"] = res
    outs = [np.asarray(res.results[c]["out"], np.float32) for c in range(8)]
    full = np.empty((B, N, DIM), np.float32)
    for b in range(B):
        full[b] = (outs[2 * b] + outs[2 * b + 1]).T + b_out[None, :]
    return full


# revision 18
# speedup vs baseline: 1.1771x; 1.0015x over previous
import sys, time

sys.path.insert(0, "/opt/trn_rl_repo")
import numpy as np
import ml_dtypes
from concourse import bass, bacc, tile, mybir
from concourse.bass_utils import run_bass_kernel_spmd

F32 = mybir.dt.float32
BF16 = mybir.dt.bfloat16
I32 = mybir.dt.int32
AL = mybir.AluOpType

B, N, DIM = 4, 2048, 1024
HEADS, DH = 16, 64
G = 8          # heads per core
GI = G * DH    # 512 = inner width per core
SCALE = DH ** -0.5
NB = N // 128   # 16 j-blocks
NCH = N // 512  # 4 q-chunks
DT = DIM // 128  # 8 dim tiles
IT = GI // 128   # 4 inner tiles

_CACHE = {}


def _build():
    nc = bacc.Bacc(None, target_bir_lowering=False)
    xT = nc.declare_dram_parameter("xT", [DIM, N], BF16, isOutput=False)
    wq = nc.declare_dram_parameter("wq", [DIM, GI], BF16, isOutput=False)
    wk = nc.declare_dram_parameter("wk", [DIM, GI], BF16, isOutput=False)
    wv = nc.declare_dram_parameter("wv", [DIM, GI], BF16, isOutput=False)
    wo = nc.declare_dram_parameter("wo", [GI, DIM], BF16, isOutput=False)
    msk = nc.declare_dram_parameter("msk", [128, 2048], BF16, isOutput=False)
    ese = nc.declare_dram_parameter("ese", [97, 256], BF16, isOutput=False)
    onv = nc.declare_dram_parameter("onv", [128, G], BF16, isOutput=False)
    out = nc.declare_dram_parameter("out", [DIM, N], BF16, isOutput=True)

    with tile.TileContext(nc) as tc:
        with (
            nc.allow_low_precision(reason="attention in bf16; rel-err gate 2e-2"),
            tc.tile_pool(name="big", bufs=1) as big,
            tc.tile_pool(name="pt", bufs=4) as ptp,
            tc.tile_pool(name="st", bufs=2) as stp,
            tc.tile_pool(name="ps", bufs=2, space="PSUM") as ps,
        ):
            # ---------------- persistent SBUF ----------------
            qT = [big.tile([128, N], BF16, name=f"qT{i}", tag=f"qT{i}") for i in range(IT)]
            kT = [big.tile([128, N], BF16, name=f"kT{i}", tag=f"kT{i}") for i in range(IT)]
            vg = [big.tile([128, G * (DH + 1)], BF16, name=f"v{r}", tag=f"v{r}") for r in range(NB)]
            mask = big.tile([128, 2048], BF16, name="mask", tag="mask")
            esel = big.tile([97, 256], BF16, name="esel", tag="esel")
            ot = [[big.tile([128, 512], BF16, name=f"ot{c}_{i}", tag=f"ot{c}_{i}") for i in range(IT)]
                  for c in range(NCH)]

            p1cm = tc.tile_pool(name="p1", bufs=1)
            p1 = p1cm.__enter__()
            xTb = [p1.tile([128, N], BF16, name=f"xT{d}", tag=f"xT{d}") for d in range(DT)]
            xTt = [[xTb[d][:, rc * 512:(rc + 1) * 512] for rc in range(4)] for d in range(DT)]
            wqb = p1.tile([128, DT * GI], BF16, name="wqb", tag="wqb")
            wkb = p1.tile([128, DT * GI], BF16, name="wkb", tag="wkb")
            wvb = p1.tile([128, DT * GI], BF16, name="wvb", tag="wvb")
            wqt = [wqb[:, d * GI:(d + 1) * GI] for d in range(DT)]
            wkt = [wkb[:, d * GI:(d + 1) * GI] for d in range(DT)]
            wvt = [wvb[:, d * GI:(d + 1) * GI] for d in range(DT)]
            wob = p1.tile([128, IT * DIM], BF16, name="wob", tag="wob")
            wot = [wob[:, i * DIM:(i + 1) * DIM] for i in range(IT)]

            # warm the ACT exp table + keep the PE busy (HAM warm) while the
            # first input DMAs land
            dmy = stp.tile([32, 32], F32, name="dmy", tag="dmy")
            nc.vector.memset(dmy[:], 0.0)
            nc.scalar.activation(dmy[:], dmy[:], mybir.ActivationFunctionType.Exp)
            dmw = p1.tile([128, 512], BF16, name="dmw", tag="dmw")
            nc.vector.memset(dmw[:], 0.0)
            for w in range(28):
                pw = ps.tile([128, 512], F32, name="pmix", tag="pmix")
                nc.tensor.matmul(pw[:], dmw[:, 0:128], dmw[:], start=True, stop=True)

            # DMAs: single coalesced descriptors, all on the sync queue so the
            # ACT queue stays free for compute. Critical-path order: wk, xT,
            # wv, wq, then the rest.
            nc.sync.dma_start(wkb[:].rearrange("p (d c) -> p d c", c=GI),
                              wk[:].rearrange("(d p) c -> p d c", p=128))
            for d in range(DT):
                nc.sync.dma_start(xTb[d][:], xT[d * 128:(d + 1) * 128, :])
            nc.sync.dma_start(wvb[:].rearrange("p (d c) -> p d c", c=GI),
                              wv[:].rearrange("(d p) c -> p d c", p=128))
            nc.sync.dma_start(wqb[:].rearrange("p (d c) -> p d c", c=GI),
                              wq[:].rearrange("(d p) c -> p d c", p=128))
            for r in range(4):
                dst = vg[r][:].rearrange("p (h c) -> p h c", c=DH + 1)[:, :, DH:DH + 1]
                nc.sync.dma_start(dst, onv[:].rearrange("p (h c) -> p h c", c=1))
            nc.sync.dma_start(mask[:], msk[:])
            nc.sync.dma_start(esel[:], ese[:])
            nc.sync.dma_start(wob[:].rearrange("p (i c) -> p i c", c=DIM),
                              wo[:].rearrange("(i p) c -> p i c", p=128))
            for r in range(4, NB):
                dst = vg[r][:].rearrange("p (h c) -> p h c", c=DH + 1)[:, :, DH:DH + 1]
                nc.sync.dma_start(dst, onv[:].rearrange("p (h c) -> p h c", c=1))

            # ---------------- projections ----------------
            evac_flip = [0]

            def evac2(dst, src):
                # alternate PSUM evacuations between DVE and ACT (phase 1 only)
                if evac_flip[0] % 2 == 0:
                    nc.vector.tensor_copy(dst, src)
                else:
                    nc.scalar.copy(dst, src)
                evac_flip[0] += 1

            def kproj_group(it, rc):
                pk = ps.tile([128, 512], F32, name="pmix", tag="pmix")
                for d in range(DT):
                    nc.tensor.matmul(pk[:], wkt[d][:, it * 128:(it + 1) * 128],
                                     xTt[d][rc], start=(d == 0), stop=(d == DT - 1))
                evac2(kT[it][:, rc * 512:(rc + 1) * 512], pk[:])

            def vproj_group(r):
                pv = ps.tile([128, 512], F32, name="pmix", tag="pmix")
                for d in range(DT):
                    nc.tensor.matmul(pv[:], xTb[d][:, r * 128:(r + 1) * 128],
                                     wvt[d], start=(d == 0), stop=(d == DT - 1))
                dst = vg[r][:].rearrange("p (h c) -> p h c", c=DH + 1)[:, :, 0:DH]
                nc.vector.tensor_copy(dst, pv[:].rearrange("p (h c) -> p h c", c=DH))

            def qproj_group(it, rc, dve_only=False):
                pq = ps.tile([128, 512], F32, name="pmix", tag="pmix")
                for d in range(DT):
                    nc.tensor.matmul(pq[:], wqt[d][:, it * 128:(it + 1) * 128],
                                     xTt[d][rc], start=(d == 0), stop=(d == DT - 1))
                if dve_only:
                    nc.vector.tensor_copy(qT[it][:, rc * 512:(rc + 1) * 512], pq[:])
                else:
                    evac2(qT[it][:, rc * 512:(rc + 1) * 512], pq[:])

            # ---------------- out-projection (transposed) ----------------
            def outproj_group(ch, db):
                # out^T[db-block, i-chunk]: lhsT is the stable weight tile so
                # the PE's LDWEIGHTS pull-ahead never reads freshly-written data
                pf = ps.tile([128, 512], F32, name="pmix", tag="pmix")
                for i in range(IT):
                    nc.tensor.matmul(pf[:], wot[i][:, db * 128:(db + 1) * 128],
                                     ot[ch][i][:],
                                     start=(i == 0), stop=(i == IT - 1))
                so = stp.tile([128, 512], BF16, name="so", tag="so")
                if db % 2 == 0:
                    nc.vector.tensor_copy(so[:], pf[:])
                else:
                    nc.scalar.copy(so[:], pf[:])
                q_ = nc.sync if db % 2 == 0 else nc.scalar
                q_.dma_start(
                    out[db * 128:(db + 1) * 128,
                        ch * 512:(ch + 1) * 512], so[:])

            # ---------------- attention ----------------
            def attention_chunk(ch):
                ej = 4 * (ch + 1)
                ns = ej // 2
                d4a = stp.tile([97, 512], F32, name="d4", tag="d4")
                d4b = stp.tile([97, 512], F32, name="d4", tag="d4")
                nc.vector.memset(d4a[:], 1.0)
                nc.vector.memset(d4b[:], 1.0)
                d4 = [d4a, d4a, d4b, d4b]

                for hp in range(4):
                    hA, hB = 2 * hp, 2 * hp + 1
                    po0 = ps.tile([65, 512], F32, name="pot", tag="pot")
                    po1 = ps.tile([65, 512], F32, name="pot", tag="pot")

                    def s_mm(s):
                        # S^T pair for a 2-block supertile: row-tiled K=64 MMs
                        supA = ps.tile([128, 1024], F32, name="sup", tag="sup")
                        supB = ps.tile([128, 1024], F32, name="sup", tag="sup")
                        for half, jb in ((0, 2 * s), (1, 2 * s + 1)):
                            sl = slice(512 * half, 512 * half + 512)
                            nc.tensor.matmul(
                                supA[:, sl], kT[hp][0:64, jb * 128:(jb + 1) * 128],
                                qT[hp][0:64, ch * 512:(ch + 1) * 512])
                            nc.tensor.matmul(
                                supB[:, sl], kT[hp][64:128, jb * 128:(jb + 1) * 128],
                                qT[hp][64:128, ch * 512:(ch + 1) * 512])
                        return supA, supB

                    nxt = s_mm(0)
                    for s in range(ns):
                        supA, supB = nxt
                        ptA = ptp.tile([128, 1024], BF16, name="pt", tag="pt")
                        ptB = ptp.tile([128, 1024], BF16, name="pt", tag="pt")
                        nc.scalar.activation(ptA[:], supA[:],
                                             mybir.ActivationFunctionType.Exp)
                        nc.scalar.activation(ptB[:], supB[:],
                                             mybir.ActivationFunctionType.Exp)
                        if s + 1 < ns:
                            nxt = s_mm(s + 1)
                        if s >= ns - 2:  # diagonal supers: staircase mask
                            msl = slice(0, 1024) if s == ns - 2 else slice(1024, 2048)
                            nc.vector.tensor_mul(ptA[:], ptA[:], mask[:, msl])
                            nc.vector.tensor_mul(ptB[:], ptB[:], mask[:, msl])
                        for half, jb in ((0, 2 * s), (1, 2 * s + 1)):
                            sl = slice(512 * half, 512 * half + 512)
                            st_ = (s == 0 and half == 0)
                            sp_ = (s == ns - 1 and half == 1)
                            nc.tensor.matmul(
                                po0[0:65, :], vg[jb][:, hA * (DH + 1):(hA + 1) * (DH + 1)],
                                ptA[:, sl], start=st_, stop=sp_)
                            nc.tensor.matmul(
                                po1[0:65, :], vg[jb][:, hB * (DH + 1):(hB + 1) * (DH + 1)],
                                ptB[:, sl], start=st_, stop=sp_)

                    # D rows -> 32-aligned slots; O -> ot (unnormalized)
                    rA, rB = 32 * (hA % 4), 32 * (hB % 4)
                    nc.vector.tensor_copy(d4[hp][rA:rA + 1, :], po0[64:65, :])
                    nc.vector.tensor_copy(d4[hp][rB:rB + 1, :], po1[64:65, :])
                    nc.vector.tensor_copy(ot[ch][hp][0:64, :], po0[0:64, :])
                    nc.vector.tensor_copy(ot[ch][hp][64:128, :], po1[0:64, :])
                    if ch + 1 < NCH:
                        qproj_group(hp, ch + 1, dve_only=True)

                # ---- batched norm for this chunk ----
                for x in range(2):
                    dd = d4a if x == 0 else d4b
                    y0 = stp.tile([97, 512], F32, name="y0", tag="y0")
                    nc.vector.reciprocal_approx_fast(y0[:], dd[:])
                    rec = stp.tile([97, 512], BF16, name="rec", tag="rec")
                    nc.vector.tensor_copy(rec[:], y0[:])
                    for t in range(2):
                        ti = 2 * x + t
                        prep = ps.tile([128, 512], F32, name="pmix", tag="pmix")
                        nc.tensor.matmul(prep[:], esel[:, 128 * t:128 * (t + 1)],
                                         rec[:], start=True, stop=True)
                        nc.vector.tensor_mul(ot[ch][ti][:], ot[ch][ti][:], prep[:])

                # out-proj: filler for the next chunk's ACT-bound attention
                for db in range(DT):
                    outproj_group(ch, db)

            # phase 1 minimal prefix: everything attention ch0 needs
            for it in range(IT):
                kproj_group(it, 0)
            for r in range(4):
                vproj_group(r)
            for it in range(IT):
                qproj_group(it, 0)

            attention_chunk(0)

            # remaining projections: emitted after ch0 so they fill its
            # ACT-bound windows; all complete before the chunks that need them
            for rc in range(1, 4):
                for it in range(IT):
                    kproj_group(it, rc)
            for r in range(4, NB):
                vproj_group(r)

            for ch in range(1, NCH):
                attention_chunk(ch)

            p1cm.__exit__(None, None, None)

    nc.compile()
    return nc


def kernel(x, w_qkv, w_out, b_out):
    if "nc" not in _CACHE:
        _CACHE["nc"] = _build()
    nc = _CACHE["nc"]

    x = np.asarray(x, np.float32)
    w_qkv = np.asarray(w_qkv, np.float32)
    w_out = np.asarray(w_out, np.float32)
    b_out = np.asarray(b_out, np.float32)

    # staircase masks for the 4 diagonal block offsets:
    # mask_r[p, i] = 1 if p <= i - 128r ; layout [r0 | r1 | r2 | r3]
    p = np.arange(128)[:, None]
    i = np.arange(512)[None, :]
    msk2 = np.concatenate(
        [(p <= i - 128 * r).astype(np.float32) for r in range(4)], axis=1)

    # selector for denominator broadcast: [97, 256]
    ese = np.zeros((97, 256), np.float32)
    ese[0, 0:64] = 1.0      # even ti: head rows 0 (p<64), 32 (p>=64)
    ese[32, 64:128] = 1.0
    ese[64, 128:192] = 1.0  # odd ti: rows 64, 96
    ese[96, 192:256] = 1.0

    in_maps = []
    for c in range(8):
        b, g = c // 2, c % 2
        sl = slice(g * GI, (g + 1) * GI)
        in_maps.append(dict(
            xT=np.ascontiguousarray(x[b].T).astype(ml_dtypes.bfloat16),
            wq=(np.ascontiguousarray(w_qkv[:, sl]) * np.float32(SCALE)).astype(ml_dtypes.bfloat16),
            wk=np.ascontiguousarray(w_qkv[:, 1024 + g * GI:1024 + (g + 1) * GI]).astype(ml_dtypes.bfloat16),
            wv=np.ascontiguousarray(w_qkv[:, 2048 + g * GI:2048 + (g + 1) * GI]).astype(ml_dtypes.bfloat16),
            wo=np.ascontiguousarray(w_out[sl, :]).astype(ml_dtypes.bfloat16),
            msk=msk2.astype(ml_dtypes.bfloat16),
            ese=ese.astype(ml_dtypes.bfloat16),
            onv=np.ones((128, G), ml_dtypes.bfloat16),
        ))
    res = None
    for attempt in range(3):
        try:
            run_bass_kernel_spmd(nc, in_maps, core_ids=list(range(8)))  # warmup
            res = run_bass_kernel_spmd(nc, in_maps, core_ids=list(range(8)))
            break
        except Exception:
            if attempt == 2:
                raise
            time.sleep(10)
    _CACHE["res"] = res
    outs = [np.asarray(res.results[c]["out"], np.float32) for c in range(8)]
    full = np.empty((B, N, DIM), np.float32)
    for b in range(B):
        full[b] = (outs[2 * b] + outs[2 * b + 1]).T + b_out[None, :]
    return full
